# revision 1
# baseline (speedup 1.0000x reference)
"""GPRGNN kernel for 8 Trainium2 NeuronCores (Bass/Tile).

Algorithm notes:
  reference: h0 = MLP(x); hidden = sum_k temp[k] * (D^-1/2 A D^-1/2)^k h0
  We propagate in g-space: g = D^-1/2 h. Then
     g_{k+1} = D^-1 * (A @ g_k)        (A = adjacency + self loops, unit weights)
     hidden  = D^1/2 * sum_k temp[k] g_k
  so per-edge norm weights vanish; each hop is a pure gather + segment-sum.

Sharding: nodes are permuted so core c owns 12544 destination slots
(12500 real nodes padded to 98 groups of 128). Nodes are assigned
round-robin by degree rank, and sorted by degree within a core, so the
128 dst nodes of a group have nearly identical in-degree -> the per-group
edge matrix [128, S_g] has ~no padding. Each hop:
  per group: one indirect DMA gathers h[src] for all edges of 128 dst
  nodes from the full replicated h in DRAM, a log-tree of vector adds
  segment-sums the slots, cheap per-partition scales produce g_{k+1},
  then an AllGather replicates all cores' new slices.
"""

import os
import sys

for _p in ("/opt/trn_rl_repo", "/opt/pypackages"):
    if _p not in sys.path:
        sys.path.insert(0, _p)

import numpy as np

N = 100_000
E = 3_200_000
F_IN = 512
H = 256
C = 64
K = 10
NCORES = 8
P = 128
G = 98                  # groups of 128 dst nodes per core
PC = G * P              # 12544 owned slots per core
NPAD = NCORES * PC      # 100352

_profile_info = {}      # filled when KERNEL_TRACE=1 (for test.py)


def _host_prep(x, edge_index):
    """Permute nodes, partition+pad edges, build per-core arrays."""
    src = np.asarray(edge_index[0], dtype=np.int64)
    dst = np.asarray(edge_index[1], dtype=np.int64)

    deg = np.bincount(dst, minlength=N).astype(np.int64) + 1  # incl self loop
    order = np.argsort(deg, kind="stable")          # ascending degree
    ranks = np.arange(N, dtype=np.int64)
    core_of = ranks % NCORES
    local_of = ranks // NCORES
    new_id = np.empty(N, dtype=np.int64)
    new_id[order] = core_of * PC + local_of         # old id -> padded new id

    ns = new_id[src]
    nd = new_id[dst]
    all_src = np.concatenate([ns, new_id])          # + self loops
    all_dst = np.concatenate([nd, new_id])
    o = np.argsort(all_dst, kind="stable")
    s_sorted = np.ascontiguousarray(all_src[o])

    deg_new = np.bincount(all_dst, minlength=NPAD).astype(np.int64)
    S_g = deg_new.reshape(NCORES, G, P).max(axis=(0, 2)).astype(np.int64)  # [G]
    S_max = int(S_g.max())
    offs = np.concatenate([[0], np.cumsum(S_g)]).astype(np.int64)
    sum_s = int(offs[-1])

    cum = np.concatenate([[0], np.cumsum(deg_new)]).astype(np.int64)
    # big[nid, j] = j-th src of node nid (pad -> owner's dummy slot, deg 0)
    zrow = (np.arange(NPAD, dtype=np.int64) // PC) * PC + (PC - 1)
    jj = np.arange(S_max, dtype=np.int64)[None, :]
    pos = np.minimum(cum[:-1][:, None] + jj, len(s_sorted) - 1)
    valid = jj < deg_new[:, None]
    big = np.where(valid, s_sorted[pos], zrow[:, None]).astype(np.int32)

    idx_blobs, xts, dinv_cols, dinv2_cols, sqd_cols = [], [], [], [], []
    deg_f = deg_new.astype(np.float64)
    with np.errstate(divide="ignore"):
        dinv_all = np.where(deg_new > 0, 1.0 / np.sqrt(np.maximum(deg_f, 1e-12)), 0.0)
        dinv2_all = np.where(deg_new > 0, 1.0 / np.maximum(deg_f, 1e-12), 0.0)
        sqd_all = np.where(deg_new > 0, np.sqrt(deg_f), 0.0)

    for c in range(NCORES):
        rows = slice(c * PC, (c + 1) * PC)
        blob = np.empty((P, sum_s), dtype=np.int32)
        bc = big[rows]                               # [PC, S_max]
        for g in range(G):
            blk = bc[g * P:(g + 1) * P, : S_g[g]]    # [128, S_g]
            blob[:, offs[g]:offs[g + 1]] = blk
        idx_blobs.append(blob)

        own_old = order[ranks[core_of == c]]         # old ids, local order asc
        xt = np.zeros((F_IN, PC), dtype=np.float32)
        xt[:, : len(own_old)] = x[own_old].T
        xts.append(np.ascontiguousarray(xt))

        dinv_cols.append(np.ascontiguousarray(
            dinv_all[rows].reshape(G, P).T.astype(np.float32)))   # [128, G]
        dinv2_cols.append(np.ascontiguousarray(
            dinv2_all[rows].reshape(G, P).T.astype(np.float32)))
        sqd_cols.append(np.ascontiguousarray(
            sqd_all[rows].reshape(G, P).T.astype(np.float32)))

    return (new_id, S_g, offs, sum_s, idx_blobs, xts,
            dinv_cols, dinv2_cols, sqd_cols)


def _build_program(S_g, offs, sum_s, temps):
    import concourse.bass as bass
    import concourse.bacc as bacc
    import concourse.mybir as mybir
    import concourse.tile as tile
    from concourse.masks import make_identity

    f32 = mybir.dt.float32
    i32 = mybir.dt.int32
    AF = mybir.ActivationFunctionType

    nc = bacc.Bacc(None, num_devices=NCORES)

    xt_d = nc.dram_tensor("xt", [F_IN, PC], f32, kind="ExternalInput")
    w1t_d = nc.dram_tensor("w1t", [F_IN, H], f32, kind="ExternalInput")
    b1_d = nc.dram_tensor("b1", [H], f32, kind="ExternalInput")
    w2t_d = nc.dram_tensor("w2t", [H, C], f32, kind="ExternalInput")
    b2_d = nc.dram_tensor("b2", [C], f32, kind="ExternalInput")
    dinv_d = nc.dram_tensor("dinv", [P, G], f32, kind="ExternalInput")
    dinv2_d = nc.dram_tensor("dinv2", [P, G], f32, kind="ExternalInput")
    sqd_d = nc.dram_tensor("sqd", [P, G], f32, kind="ExternalInput")
    idx_d = nc.dram_tensor("idx", [P, sum_s], i32, kind="ExternalInput")
    outl_d = nc.dram_tensor("outl", [PC, C], f32, kind="ExternalOutput")

    own_d = nc.dram_tensor("own", [PC, C], f32)
    ha_d = nc.dram_tensor("ha", [NPAD, C], f32, addr_space="Shared")
    hb_d = nc.dram_tensor("hb", [NPAD, C], f32, addr_space="Shared")
    debug_ha = os.environ.get("KERNEL_DEBUG_HA", "0") == "1"
    if debug_ha:
        hdbg_d = nc.dram_tensor("hdbg", [NPAD, C], f32, kind="ExternalOutput")

    groups = [list(range(NCORES))]

    with tile.TileContext(nc) as tc:
        with (
            tc.tile_pool(name="const", bufs=1) as cpool,
            tc.tile_pool(name="xin", bufs=3) as xpool,
            tc.tile_pool(name="mlp", bufs=3) as mpool,
            tc.tile_pool(name="gat", bufs=3) as gpool,
            tc.tile_pool(name="small", bufs=4) as spool,
            tc.tile_pool(name="ps", bufs=2, space="PSUM") as ppool,
            tc.tile_pool(name="ps2", bufs=2, space="PSUM") as ppool2,
        ):
            # ---- constants / persistent state ----
            w1t_sb = cpool.tile([P, 4 * H], f32)      # [128, (kc, 256)]
            nc.sync.dma_start(
                w1t_sb[:].rearrange("p (kc h) -> p kc h", kc=4),
                w1t_d[:].rearrange("(kc p) h -> p kc h", p=P))
            w2t_sb = cpool.tile([P, 2 * C], f32)      # [128, (jc, 64)]
            nc.sync.dma_start(
                w2t_sb[:].rearrange("p (jc c) -> p jc c", jc=2),
                w2t_d[:].rearrange("(jc p) c -> p jc c", p=P))
            b1_sb = cpool.tile([P, 2], f32)
            nc.sync.dma_start(b1_sb[:], b1_d[:].rearrange("(jc p) -> p jc", p=P))
            b2_sb = cpool.tile([P, 1], f32)
            nc.sync.dma_start(b2_sb[:C, :], b2_d[:].rearrange("(c one) -> c one", one=1))
            dinv_sb = cpool.tile([P, G], f32)
            nc.sync.dma_start(dinv_sb[:], dinv_d[:])
            dinv2_sb = cpool.tile([P, G], f32)
            nc.sync.dma_start(dinv2_sb[:], dinv2_d[:])
            sqd_sb = cpool.tile([P, G], f32)
            nc.sync.dma_start(sqd_sb[:], sqd_d[:])
            idx_sb = cpool.tile([P, sum_s], i32)
            nc.sync.dma_start(idx_sb[:], idx_d[:])
            ident = cpool.tile([P, P], f32)
            make_identity(nc, ident[:])
            hidden = cpool.tile([P, G * C], f32)

            # ---- phase A: MLP + g0 ----
            for g in range(G):
                xt_sb = xpool.tile([P, 4, P], f32, tag="xt")
                nc.sync.dma_start(
                    xt_sb[:],
                    xt_d[:, g * P:(g + 1) * P].rearrange(
                        "(kc p) n -> p kc n", p=P))
                h1_sb = mpool.tile([P, 2 * P], f32, tag="h1")
                for jc in range(2):
                    ps1 = ppool.tile([P, P], f32, tag="ps1")
                    for kc in range(4):
                        nc.tensor.matmul(
                            ps1[:],
                            lhsT=w1t_sb[:, kc * H + jc * P: kc * H + (jc + 1) * P],
                            rhs=xt_sb[:, kc, :],
                            start=(kc == 0), stop=(kc == 3))
                    nc.scalar.activation(
                        h1_sb[:, jc * P:(jc + 1) * P], ps1[:],
                        AF.Relu, bias=b1_sb[:, jc:jc + 1])
                ps2 = ppool.tile([P, P], f32, tag="ps2")
                for jc in range(2):
                    nc.tensor.matmul(
                        ps2[:C, :],
                        lhsT=w2t_sb[:, jc * C:(jc + 1) * C],
                        rhs=h1_sb[:, jc * P:(jc + 1) * P],
                        start=(jc == 0), stop=(jc == 1))
                h2_sb = mpool.tile([P, P], f32, tag="h2")
                nc.scalar.activation(h2_sb[:C, :], ps2[:C, :],
                                     AF.Identity, bias=b2_sb[:C, :])
                pst = ppool2.tile([P, C], f32, tag="pst")
                nc.tensor.transpose(pst[:], h2_sb[:C, :], ident[:C, :C])
                g0_sb = spool.tile([P, C], f32, tag="gn")
                nc.vector.tensor_scalar_mul(g0_sb[:], pst[:], dinv_sb[:, g:g + 1])
                nc.scalar.mul(hidden[:, g * C:(g + 1) * C], g0_sb[:], float(temps[0]))
                nc.sync.dma_start(own_d[g * P:(g + 1) * P, :], g0_sb[:])

            nc.gpsimd.collective_compute(
                "AllGather", mybir.AluOpType.bypass, replica_groups=groups,
                ins=[own_d[:]], outs=[ha_d[:]])
            if debug_ha and K == 1:
                nc.sync.dma_start(hdbg_d[:], ha_d[:])

            # ---- phase B: K hops ----
            hcur, hnxt = ha_d, hb_d
            for k in range(K):
                tk = float(temps[k + 1])
                for g in range(G):
                    S = int(S_g[g])
                    off = int(offs[g])
                    gbuf = gpool.tile([P, S * C], f32, tag="gbuf")
                    nc.gpsimd.indirect_dma_start(
                        out=gbuf[:],
                        out_offset=None,
                        in_=hcur[:],
                        in_offset=bass.IndirectOffsetOnAxis(
                            ap=idx_sb[:, off:off + S], axis=0))
                    # log-tree fold: keep first ceil(S/2) slots
                    s = S
                    while s > 1:
                        h_ = s // 2
                        nc.vector.tensor_add(
                            gbuf[:, : h_ * C],
                            gbuf[:, : h_ * C],
                            gbuf[:, (s - h_) * C: s * C])
                        s -= h_
                    gn = spool.tile([P, C], f32, tag="gn")
                    nc.vector.tensor_scalar_mul(
                        gn[:], gbuf[:, :C], dinv2_sb[:, g:g + 1])
                    tmp = spool.tile([P, C], f32, tag="tmp")
                    nc.scalar.mul(tmp[:], gn[:], tk)
                    nc.vector.tensor_add(
                        hidden[:, g * C:(g + 1) * C],
                        hidden[:, g * C:(g + 1) * C], tmp[:])
                    if k < K - 1:
                        nc.sync.dma_start(own_d[g * P:(g + 1) * P, :], gn[:])
                if k < K - 1:
                    nc.gpsimd.collective_compute(
                        "AllGather", mybir.AluOpType.bypass,
                        replica_groups=groups,
                        ins=[own_d[:]], outs=[hnxt[:]])
                    if debug_ha and k == 0:
                        nc.sync.dma_start(hdbg_d[:], hnxt[:])
                    hcur, hnxt = hnxt, hcur

            # ---- phase C: hidden * sqrt(deg), log_softmax, store ----
            for g in range(G):
                hid = spool.tile([P, C], f32, tag="hid")
                nc.vector.tensor_scalar_mul(
                    hid[:], hidden[:, g * C:(g + 1) * C], sqd_sb[:, g:g + 1])
                nm = spool.tile([P, 1], f32, tag="nm")
                nc.vector.reduce_max(nm[:], hid[:], axis=mybir.AxisListType.X,
                                     negate=True)
                ex = spool.tile([P, C], f32, tag="ex")
                nc.scalar.activation(ex[:], hid[:], AF.Exp, bias=nm[:, 0:1])
                ssum = spool.tile([P, 1], f32, tag="ssum")
                nc.vector.reduce_sum(ssum[:], ex[:], axis=mybir.AxisListType.X)
                lse = spool.tile([P, 1], f32, tag="lse")
                nc.scalar.activation(lse[:], ssum[:], AF.Ln)
                c1 = spool.tile([P, 1], f32, tag="c1")
                nc.vector.tensor_tensor(
                    out=c1[:], in0=nm[:], in1=lse[:],
                    op=mybir.AluOpType.subtract)
                o_sb = spool.tile([P, C], f32, tag="o")
                nc.vector.tensor_scalar_add(o_sb[:], hid[:], c1[:, 0:1])
                nc.sync.dma_start(outl_d[g * P:(g + 1) * P, :], o_sb[:])

    nc.finalize()
    return nc


def kernel(x, w1, b1, w2, b2, temp, edge_index):
    from concourse.bass_utils import run_bass_kernel_spmd

    x = np.asarray(x, dtype=np.float32)
    w1 = np.asarray(w1, dtype=np.float32)
    b1 = np.asarray(b1, dtype=np.float32)
    w2 = np.asarray(w2, dtype=np.float32)
    b2 = np.asarray(b2, dtype=np.float32)
    temp = np.asarray(temp, dtype=np.float32)

    (new_id, S_g, offs, sum_s, idx_blobs, xts,
     dinv_cols, dinv2_cols, sqd_cols) = _host_prep(x, edge_index)

    nc = _build_program(S_g, offs, sum_s, [float(t) for t in temp])

    w1t = np.ascontiguousarray(w1.T)          # [512, 256]
    w2t = np.ascontiguousarray(w2.T)          # [256, 64]
    in_maps = []
    for c in range(NCORES):
        in_maps.append({
            "xt": xts[c],
            "w1t": w1t, "b1": b1, "w2t": w2t, "b2": b2,
            "dinv": dinv_cols[c], "dinv2": dinv2_cols[c], "sqd": sqd_cols[c],
            "idx": idx_blobs[c],
        })

    trace = os.environ.get("KERNEL_TRACE", "0") == "1"
    res = run_bass_kernel_spmd(nc, in_maps, list(range(NCORES)), trace=trace)
    if trace:
        _profile_info["exec_time_ns"] = res.exec_time_ns
        _profile_info["mean_exec_time_ns"] = res.mean_exec_time_ns
        _profile_info["profile_json"] = res.profile_json

    full = np.concatenate([res.results[c]["outl"] for c in range(NCORES)], axis=0)
    return np.ascontiguousarray(full[new_id])



# revision 20
# speedup vs baseline: 2.0226x; 2.0226x over previous
"""GPRGNN kernel for 8 Trainium2 NeuronCores (Bass/Tile).

Algorithm notes:
  reference: h0 = MLP(x); hidden = sum_k temp[k] * (D^-1/2 A D^-1/2)^k h0
  We propagate in g-space: g = D^-1/2 h. Then
     g_{k+1} = D^-1 * (A @ g_k)        (A = adjacency + self loops, unit weights)
     hidden  = D^1/2 * sum_k temp[k] g_k
  so per-edge norm weights vanish; each hop is a pure gather + segment-sum.

Sharding: nodes are permuted so core c owns 12544 destination slots
(12500 real nodes padded to 98 groups of 128). Nodes are assigned
round-robin by degree rank, and sorted by degree within a core, so the
128 dst nodes of a group have nearly identical in-degree.

v1 layout (fp16 propagation):
  - The replicated hop table ha/hb is fp16 [NPAD, C]; per-hop gather
    traffic halves vs fp32.
  - Consecutive groups with EQUAL padded slot count S are batched (B
    groups per batch, group-major gbuf layout col = b*S + s); the gather
    is one indirect DMA per group (matching the toolchain's lowering of
    the indirect offset AP), while the log-tree segment-sum folds, the
    1/deg scale, and the hidden accumulation run once per batch as
    strided fp16 tensor ops (2x DVE mode).
  - Each core's newly computed slice lives in SBUF as gall [128, G*C]
    (partition-major), written back per AllGather chunk with ONE large
    contiguous DMA; the AllGather output therefore has rows keyed
    (core, partition, group) and the host builds gather indices for that
    layout directly.
  - The per-hop AllGather is split in two chunks (groups [0,GS) and
    [GS,G)); chunk0's AllGather overlaps the tail groups' gather+fold
    work, so only the small chunk1 AllGather is exposed on the critical
    path.
  - MLP runs in fp16 (PSUM accumulation in fp32), weights/x cast on host.
  - log_softmax runs batched over all groups at once; output is stored
    partition-major [P, G*C] and unsharded on host.
"""

import os
import sys

for _p in ("/opt/trn_rl_repo", "/opt/pypackages"):
    if _p not in sys.path:
        sys.path.insert(0, _p)

import numpy as np

N = 100_000
E = 3_200_000
F_IN = 512
H = 256
C = 64
K = 10
NCORES = 8
P = 128
G = 98                  # groups of 128 dst nodes per core
PC = G * P              # 12544 owned slots per core
NPAD = NCORES * PC      # 100352
GS = 70                 # AllGather chunk split: groups [0,GS) | [GS,G)
SLOT_CAP = 224          # max padded slots (B*S) per batch (group-major)
ROWS0 = NCORES * GS * P         # table rows in chunk 0
G1 = G - GS

_profile_info = {}      # filled when KERNEL_TRACE=1 (for test.py)


def _table_row(core, g, p):
    """Row in the replicated hop table for node (core, group, partition).

    Chunk c0 (g < GS): AllGather of [P, GS*C] blobs -> rank-major rows of
    (p, g) pairs. Chunk c1 analogous, offset by ROWS0.
    """
    in0 = g < GS
    r0 = core * (GS * P) + p * GS + g
    r1 = ROWS0 + core * (G1 * P) + p * G1 + (g - GS)
    return np.where(in0, r0, r1)


def _host_prep(x, w1, w2, edge_index):
    import ml_dtypes

    src = np.asarray(edge_index[0], dtype=np.int64)
    dst = np.asarray(edge_index[1], dtype=np.int64)

    deg = np.bincount(dst, minlength=N).astype(np.int64) + 1  # incl self loop
    order = np.argsort(deg, kind="stable")          # ascending degree
    ranks = np.arange(N, dtype=np.int64)
    core_r = ranks % NCORES
    local_r = ranks // NCORES
    new_id = np.empty(N, dtype=np.int64)
    new_id[order] = core_r * PC + local_r           # old id -> padded new id

    g_r = local_r // P
    p_r = local_r % P
    trow_r = _table_row(core_r, g_r, p_r)           # rank -> table row
    trow_old = np.empty(N, dtype=np.int64)
    trow_old[order] = trow_r

    loop = np.arange(N, dtype=np.int64)
    all_dst = new_id[np.concatenate([dst, loop])]
    all_srcrow = trow_old[np.concatenate([src, loop])]
    o = np.lexsort((all_srcrow, all_dst))           # by dst, then src row asc
    s_sorted = np.ascontiguousarray(all_srcrow[o])

    deg_new = np.bincount(all_dst, minlength=NPAD).astype(np.int64)
    S_g = deg_new.reshape(NCORES, G, P).max(axis=(0, 2)).astype(np.int64)  # [G]

    # batch plan: consecutive groups at uniform pitch Smax = max S_g in the
    # batch, B*Smax <= SLOT_CAP, no batch crosses GS. Fetches stay exactly
    # S_g wide per group; the gbuf gap [S_g, Smax) is zeroed on device.
    batches = []           # (g0, B, Smax)
    g0 = 0
    while g0 < G:
        lim = GS if g0 < GS else G
        B = 1
        S = int(S_g[g0])
        while g0 + B < lim:
            S2 = max(S, int(S_g[g0 + B]))
            if (B + 1) * S2 > SLOT_CAP:
                break
            B += 1
            S = S2
        batches.append((g0, B, S))
        g0 += B
    S_list = [int(s) for s in S_g]
    total_cols = int(S_g.sum())

    # dummy zero row: core 0's last pad slot (deg 0 -> value always 0)
    dummy = int(_table_row(np.int64(0), np.int64(G - 1), np.int64(P - 1)))

    cum = np.concatenate([[0], np.cumsum(deg_new)]).astype(np.int64)
    S_max = int(S_g.max())
    jj = np.arange(S_max, dtype=np.int64)[None, :]
    pos = np.minimum(cum[:-1][:, None] + jj, len(s_sorted) - 1)
    valid = jj < deg_new[:, None]
    big = np.where(valid, s_sorted[pos], dummy).astype(np.int32)  # [NPAD, S_max]

    deg_f = deg_new.astype(np.float64)
    with np.errstate(divide="ignore"):
        dinv_all = np.where(deg_new > 0, 1.0 / np.sqrt(np.maximum(deg_f, 1e-12)), 0.0)
        dinv2_all = np.where(deg_new > 0, 1.0 / np.maximum(deg_f, 1e-12), 0.0)
        sqd_all = np.where(deg_new > 0, np.sqrt(deg_f), 0.0)

    bf16 = ml_dtypes.bfloat16
    idx_blobs, xts, dinvs, dinv2xs, sqdxs = [], [], [], [], []
    for c in range(NCORES):
        rows = slice(c * PC, (c + 1) * PC)
        bc = big[rows].reshape(G, P, S_max)
        blob = np.empty((P, total_cols), dtype=np.int32)
        off = 0
        for g in range(G):
            Sg = S_list[g]
            blob[:, off:off + Sg] = bc[g, :, :Sg]
            off += Sg
        idx_blobs.append(np.ascontiguousarray(blob))

        own_old = order[ranks[core_r == c]]          # old ids, local order asc
        xt = np.zeros((F_IN, PC), dtype=np.float32)
        xt[:, : len(own_old)] = x[own_old].T
        xts.append(np.ascontiguousarray(xt.astype(np.float16)))

        dinvs.append(np.ascontiguousarray(
            dinv_all[rows].reshape(G, P).T.astype(np.float32)))   # [128, G]
        d2 = dinv2_all[rows].reshape(G, P).T.astype(np.float16)   # [128, G]
        dinv2xs.append(np.ascontiguousarray(np.repeat(d2, C, axis=1)))
        sq = sqd_all[rows].reshape(G, P).T.astype(np.float16)
        sqdxs.append(np.ascontiguousarray(np.repeat(sq, C, axis=1)))

    w1t = np.ascontiguousarray(w1.T.astype(np.float16))    # [512, 256]
    w2t = np.ascontiguousarray(w2.T.astype(np.float16))    # [256, 64]

    return (new_id, batches, S_list, total_cols, idx_blobs, xts,
            dinvs, dinv2xs, sqdxs, w1t, w2t)


def _build_program(batches, S_list, total_cols, temps):
    import concourse.bass as bass
    import concourse.bacc as bacc
    import concourse.mybir as mybir
    import concourse.tile as tile
    from concourse.bass import broadcast_tensor_aps
    from concourse.masks import make_identity

    f32 = mybir.dt.float32
    f16 = mybir.dt.float16
    bf16 = mybir.dt.bfloat16
    i32 = mybir.dt.int32
    AF = mybir.ActivationFunctionType
    ALU = mybir.AluOpType

    maxB = max(B for (_, B, _) in batches)
    maxSB = max(B * S for (_, B, S) in batches)

    nc = bacc.Bacc(None, num_devices=NCORES)

    xt_d = nc.dram_tensor("xt", [F_IN, PC], f16, kind="ExternalInput")
    w1t_d = nc.dram_tensor("w1t", [F_IN, H], f16, kind="ExternalInput")
    b1_d = nc.dram_tensor("b1", [H], f32, kind="ExternalInput")
    w2t_d = nc.dram_tensor("w2t", [H, C], f16, kind="ExternalInput")
    b2_d = nc.dram_tensor("b2", [C], f32, kind="ExternalInput")
    dinv_d = nc.dram_tensor("dinv", [P, G], f32, kind="ExternalInput")
    dinv2x_d = nc.dram_tensor("dinv2x", [P, G * C], f16, kind="ExternalInput")
    sqdx_d = nc.dram_tensor("sqdx", [P, G * C], f16, kind="ExternalInput")
    idx_d = nc.dram_tensor("idx", [P, total_cols], i32, kind="ExternalInput")
    outl_d = nc.dram_tensor("outl", [P, G * C], f32, kind="ExternalOutput")

    own0_d = nc.dram_tensor("own0", [P, GS * C], f16)
    own1_d = nc.dram_tensor("own1", [P, G1 * C], f16)
    ha_d = nc.dram_tensor("ha", [NPAD, C], f16, addr_space="Shared")
    hb_d = nc.dram_tensor("hb", [NPAD, C], f16, addr_space="Shared")
    debug_dump = os.environ.get("KERNEL_DEBUG_DUMP", "0") == "1"
    if debug_dump:
        hdbg0_d = nc.dram_tensor("hdbg0", [NPAD, C], f16, kind="ExternalOutput")
        hdbg1_d = nc.dram_tensor("hdbg1", [NPAD, C], f16, kind="ExternalOutput")
    debug_gbuf = os.environ.get("KERNEL_DEBUG_GBUF", "0") == "1"
    if debug_gbuf:
        sb0 = batches[0][1] * batches[0][2] * C
        sbL = batches[-1][1] * batches[-1][2] * C
        gdbg0_d = nc.dram_tensor("gdbg0", [P, sb0], f16, kind="ExternalOutput")
        gdbgL_d = nc.dram_tensor("gdbgL", [P, sbL], f16, kind="ExternalOutput")

    groups = [list(range(NCORES))]

    with tile.TileContext(nc) as tc:
        with (
            tc.tile_pool(name="const", bufs=1) as cpool,
            tc.tile_pool(name="xin", bufs=3) as xpool,
            tc.tile_pool(name="mlp", bufs=3) as mpool,
            tc.tile_pool(name="small", bufs=4) as spool,
            tc.tile_pool(name="ps", bufs=2, space="PSUM") as ppool,
            tc.tile_pool(name="ps2", bufs=2, space="PSUM") as ppool2,
        ):
            # ---- constants / persistent state ----
            w1t_sb = cpool.tile([P, 4 * H], f16)     # [128, (kc, 256)]
            nc.sync.dma_start(
                w1t_sb[:].rearrange("p (kc h) -> p kc h", kc=4),
                w1t_d[:].rearrange("(kc p) h -> p kc h", p=P))
            w2t_sb = cpool.tile([P, 2 * C], f16)     # [128, (jc, 64)]
            nc.sync.dma_start(
                w2t_sb[:].rearrange("p (jc c) -> p jc c", jc=2),
                w2t_d[:].rearrange("(jc p) c -> p jc c", p=P))
            b1_sb = cpool.tile([P, 2], f32)
            nc.sync.dma_start(b1_sb[:], b1_d[:].rearrange("(jc p) -> p jc", p=P))
            b2_sb = cpool.tile([P, 1], f32)
            nc.sync.dma_start(b2_sb[:C, :], b2_d[:].rearrange("(c one) -> c one", one=1))
            dinv_sb = cpool.tile([P, G], f32)
            nc.sync.dma_start(dinv_sb[:], dinv_d[:])
            dinv2x_sb = cpool.tile([P, G * C], f16)
            nc.sync.dma_start(dinv2x_sb[:], dinv2x_d[:])
            sqdx_sb = cpool.tile([P, G * C], f16)
            nc.sync.dma_start(sqdx_sb[:], sqdx_d[:])
            idx_sb = cpool.tile([P, total_cols], i32)
            nc.sync.dma_start(idx_sb[:], idx_d[:])
            ident = cpool.tile([P, P], f32)
            make_identity(nc, ident[:])
            hidden = cpool.tile([P, G * C], f16)
            gall = cpool.tile([P, G * C], f16)

            def flush_chunk(chunk, dst_table):
                if chunk == 0:
                    nc.sync.dma_start(own0_d[:], gall[:, : GS * C])
                    nc.gpsimd.collective_compute(
                        "AllGather", ALU.bypass, replica_groups=groups,
                        ins=[own0_d[:]], outs=[dst_table[0:ROWS0, :]])
                else:
                    nc.sync.dma_start(own1_d[:], gall[:, GS * C:])
                    nc.gpsimd.collective_compute(
                        "AllGather", ALU.bypass, replica_groups=groups,
                        ins=[own1_d[:]], outs=[dst_table[ROWS0:, :]])

            # ---- phase A: MLP + g0 ----
            for g in range(G):
                xt_sb = xpool.tile([P, 4, P], f16, tag="xt")
                nc.sync.dma_start(
                    xt_sb[:],
                    xt_d[:, g * P:(g + 1) * P].rearrange(
                        "(kc p) n -> p kc n", p=P))
                h1_sb = mpool.tile([P, 2 * P], f16, tag="h1")
                for jc in range(2):
                    ps1 = ppool.tile([P, P], f32, tag="ps1")
                    for kc in range(4):
                        nc.tensor.matmul(
                            ps1[:],
                            lhsT=w1t_sb[:, kc * H + jc * P: kc * H + (jc + 1) * P],
                            rhs=xt_sb[:, kc, :],
                            start=(kc == 0), stop=(kc == 3))
                    nc.scalar.activation(
                        h1_sb[:, jc * P:(jc + 1) * P], ps1[:],
                        AF.Relu, bias=b1_sb[:, jc:jc + 1])
                ps2 = ppool.tile([P, P], f32, tag="ps2")
                for jc in range(2):
                    nc.tensor.matmul(
                        ps2[:C, :],
                        lhsT=w2t_sb[:, jc * C:(jc + 1) * C],
                        rhs=h1_sb[:, jc * P:(jc + 1) * P],
                        start=(jc == 0), stop=(jc == 1))
                h2_sb = mpool.tile([P, P], f32, tag="h2")
                nc.scalar.activation(h2_sb[:C, :], ps2[:C, :],
                                     AF.Identity, bias=b2_sb[:C, :])
                pst = ppool2.tile([P, C], f32, tag="pst")
                nc.tensor.transpose(pst[:], h2_sb[:C, :], ident[:C, :C])
                sl = slice(g * C, (g + 1) * C)
                nc.vector.tensor_scalar_mul(gall[:, sl], pst[:], dinv_sb[:, g:g + 1])
                nc.scalar.mul(hidden[:, sl], gall[:, sl], float(temps[0]))
                if g == GS - 1:
                    flush_chunk(0, ha_d)
            flush_chunk(1, ha_d)
            if debug_dump:
                nc.sync.dma_start(hdbg0_d[:], ha_d[:])

            # ---- phase B: K hops ----
            with tc.tile_pool(name="gat", bufs=4) as gpool:
                hcur, hnxt = ha_d, hb_d
                for k in range(K):
                    tk = float(temps[k + 1])
                    off = 0
                    for (g0, B, S) in batches:
                        gbuf = gpool.tile([P, maxSB * C], f16, tag="gbuf")
                        for b in range(B):
                            Sb = S_list[g0 + b]
                            nc.gpsimd.indirect_dma_start(
                                out=gbuf[:, b * S * C: b * S * C + Sb * C],
                                out_offset=None,
                                in_=hcur[:],
                                in_offset=bass.IndirectOffsetOnAxis(
                                    ap=idx_sb[:, off: off + Sb], axis=0))
                            if Sb < S:
                                nc.vector.memset(
                                    gbuf[:, b * S * C + Sb * C:
                                         (b + 1) * S * C], 0.0)
                            off += Sb
                        # log-tree fold over slots (group-major layout, strided)
                        gv = gbuf[:, : B * S * C].rearrange(
                            "p (b x) -> p b x", b=B)
                        s = S
                        while s > 1:
                            h_ = s // 2
                            nc.vector.tensor_add(
                                gv[:, :, : h_ * C],
                                gv[:, :, : h_ * C],
                                gv[:, :, (s - h_) * C: s * C])
                            s -= h_
                        sl = slice(g0 * C, (g0 + B) * C)
                        nc.vector.tensor_tensor(
                            out=gall[:, sl].rearrange("p (b c) -> p b c", c=C),
                            in0=gv[:, :, :C],
                            in1=dinv2x_sb[:, sl].rearrange(
                                "p (b c) -> p b c", c=C),
                            op=ALU.mult)
                        if k < K - 1:
                            if g0 + B == GS:
                                flush_chunk(0, hnxt)
                            elif g0 + B == G:
                                flush_chunk(1, hnxt)
                    # hidden += tk * g, deferred so it fills the AllGather
                    # wait window at the hop boundary (reads gall only)
                    for (g0, B, S) in batches:
                        sl = slice(g0 * C, (g0 + B) * C)
                        tmp = spool.tile([P, maxB * C], f16, tag="tmp")
                        nc.scalar.mul(tmp[:, : B * C], gall[:, sl], tk)
                        nc.vector.tensor_add(
                            hidden[:, sl], hidden[:, sl], tmp[:, : B * C])
                    if debug_dump and k == 0:
                        nc.sync.dma_start(hdbg1_d[:], hnxt[:])
                    hcur, hnxt = hnxt, hcur

            # ---- phase C: hidden * sqrt(deg), log_softmax, store ----
            with tc.tile_pool(name="smx", bufs=1) as opool:
                hidf = opool.tile([P, G * C], f32)
                nc.vector.tensor_tensor(
                    out=hidf[:], in0=hidden[:], in1=sqdx_sb[:], op=ALU.mult)
                hid3 = hidf[:].rearrange("p (g c) -> p g c", c=C)
                nm = opool.tile([P, G], f32)
                nc.vector.reduce_max(nm[:], hid3, axis=mybir.AxisListType.X,
                                     negate=True)
                nm3 = nm[:].rearrange("p (g one) -> p g one", one=1)
                h_b, nm_b = broadcast_tensor_aps(hid3, nm3)
                nc.vector.tensor_tensor(out=hid3, in0=h_b, in1=nm_b, op=ALU.add)
                exf = opool.tile([P, G * C], f16)
                nc.scalar.activation(exf[:], hidf[:], AF.Exp)
                ssum = opool.tile([P, G], f32)
                nc.vector.reduce_sum(
                    ssum[:], exf[:].rearrange("p (g c) -> p g c", c=C),
                    axis=mybir.AxisListType.X)
                lse = opool.tile([P, G], f32)
                nc.scalar.activation(lse[:], ssum[:], AF.Ln)
                lse3 = lse[:].rearrange("p (g one) -> p g one", one=1)
                osb = opool.tile([P, G * C], f32)
                h_b2, lse_b = broadcast_tensor_aps(hid3, lse3)
                nc.vector.tensor_tensor(
                    out=osb[:].rearrange("p (g c) -> p g c", c=C),
                    in0=h_b2, in1=lse_b, op=ALU.subtract)
                nc.sync.dma_start(outl_d[:], osb[:])

    nc.finalize()
    return nc


def kernel(x, w1, b1, w2, b2, temp, edge_index):
    from concourse.bass_utils import run_bass_kernel_spmd

    x = np.asarray(x, dtype=np.float32)
    w1 = np.asarray(w1, dtype=np.float32)
    b1 = np.asarray(b1, dtype=np.float32)
    w2 = np.asarray(w2, dtype=np.float32)
    b2 = np.asarray(b2, dtype=np.float32)
    temp = np.asarray(temp, dtype=np.float32)

    (new_id, batches, S_list, total_cols, idx_blobs, xts,
     dinvs, dinv2xs, sqdxs, w1t, w2t) = _host_prep(x, w1, w2, edge_index)

    nc = _build_program(batches, S_list, total_cols, [float(t) for t in temp])

    in_maps = []
    for c in range(NCORES):
        in_maps.append({
            "xt": xts[c],
            "w1t": w1t, "b1": b1, "w2t": w2t, "b2": b2,
            "dinv": dinvs[c], "dinv2x": dinv2xs[c], "sqdx": sqdxs[c],
            "idx": idx_blobs[c],
        })

    trace = os.environ.get("KERNEL_TRACE", "0") == "1"
    res = run_bass_kernel_spmd(nc, in_maps, list(range(NCORES)), trace=trace)
    if trace:
        _profile_info["exec_time_ns"] = res.exec_time_ns
        _profile_info["mean_exec_time_ns"] = res.mean_exec_time_ns
        _profile_info["profile_json"] = res.profile_json

    # outl is [P, G*C] partition-major; node (core, g, p) -> [p, g*C:(g+1)*C]
    parts = []
    for c in range(NCORES):
        o = res.results[c]["outl"].reshape(P, G, C)
        parts.append(np.ascontiguousarray(o.transpose(1, 0, 2).reshape(PC, C)))
    full = np.concatenate(parts, axis=0)
    return np.ascontiguousarray(full[new_id])


# revision 21
# speedup vs baseline: 2.0928x; 1.0347x over previous
"""GPRGNN kernel for 8 Trainium2 NeuronCores (Bass/Tile).

Algorithm notes:
  reference: h0 = MLP(x); hidden = sum_k temp[k] * (D^-1/2 A D^-1/2)^k h0
  We propagate in g-space: g = D^-1/2 h. Then
     g_{k+1} = D^-1 * (A @ g_k)        (A = adjacency + self loops, unit weights)
     hidden  = D^1/2 * sum_k temp[k] g_k
  so per-edge norm weights vanish; each hop is a pure gather + segment-sum.

Sharding: nodes are permuted so core c owns 12544 destination slots
(12500 real nodes padded to 98 groups of 128). Nodes are assigned
round-robin by degree rank, and sorted by degree within a core, so the
128 dst nodes of a group have nearly identical in-degree.

v1 layout (fp16 propagation):
  - The replicated hop table ha/hb is fp16 [NPAD, C]; per-hop gather
    traffic halves vs fp32.
  - Consecutive groups with EQUAL padded slot count S are batched (B
    groups per batch, group-major gbuf layout col = b*S + s); the gather
    is one indirect DMA per group (matching the toolchain's lowering of
    the indirect offset AP), while the log-tree segment-sum folds, the
    1/deg scale, and the hidden accumulation run once per batch as
    strided fp16 tensor ops (2x DVE mode).
  - Each core's newly computed slice lives in SBUF as gall [128, G*C]
    (partition-major), written back per AllGather chunk with ONE large
    contiguous DMA; the AllGather output therefore has rows keyed
    (core, partition, group) and the host builds gather indices for that
    layout directly.
  - The per-hop AllGather is split in two chunks (groups [0,GS) and
    [GS,G)); chunk0's AllGather overlaps the tail groups' gather+fold
    work, so only the small chunk1 AllGather is exposed on the critical
    path.
  - MLP runs in fp16 (PSUM accumulation in fp32), weights/x cast on host.
  - log_softmax runs batched over all groups at once; output is stored
    partition-major [P, G*C] and unsharded on host.
"""

import os
import sys

for _p in ("/opt/trn_rl_repo", "/opt/pypackages"):
    if _p not in sys.path:
        sys.path.insert(0, _p)

import numpy as np

N = 100_000
E = 3_200_000
F_IN = 512
H = 256
C = 64
K = 10
NCORES = 8
P = 128
G = 98                  # groups of 128 dst nodes per core
PC = G * P              # 12544 owned slots per core
NPAD = NCORES * PC      # 100352
GS = 70                 # AllGather chunk split: groups [0,GS) | [GS,G)
SLOT_CAP = 224          # max padded slots (B*S) per batch (group-major)
ROWS0 = NCORES * GS * P         # table rows in chunk 0
G1 = G - GS

_profile_info = {}      # filled when KERNEL_TRACE=1 (for test.py)


def _table_row(core, g, p):
    """Row in the replicated hop table for node (core, group, partition).

    Chunk c0 (g < GS): AllGather of [P, GS*C] blobs -> rank-major rows of
    (p, g) pairs. Chunk c1 analogous, offset by ROWS0.
    """
    in0 = g < GS
    r0 = core * (GS * P) + p * GS + g
    r1 = ROWS0 + core * (G1 * P) + p * G1 + (g - GS)
    return np.where(in0, r0, r1)


def _host_prep(x, w1, w2, edge_index):
    import ml_dtypes

    src = np.asarray(edge_index[0], dtype=np.int64)
    dst = np.asarray(edge_index[1], dtype=np.int64)

    deg = np.bincount(dst, minlength=N).astype(np.int64) + 1  # incl self loop
    order = np.argsort(deg, kind="stable")          # ascending degree
    ranks = np.arange(N, dtype=np.int64)
    core_r = ranks % NCORES
    local_r = ranks // NCORES
    new_id = np.empty(N, dtype=np.int64)
    new_id[order] = core_r * PC + local_r           # old id -> padded new id

    g_r = local_r // P
    p_r = local_r % P
    trow_r = _table_row(core_r, g_r, p_r)           # rank -> table row
    trow_old = np.empty(N, dtype=np.int64)
    trow_old[order] = trow_r

    loop = np.arange(N, dtype=np.int64)
    all_dst = new_id[np.concatenate([dst, loop])]
    all_srcrow = trow_old[np.concatenate([src, loop])]
    o = np.lexsort((all_srcrow, all_dst))           # by dst, then src row asc
    s_sorted = np.ascontiguousarray(all_srcrow[o])

    deg_new = np.bincount(all_dst, minlength=NPAD).astype(np.int64)
    S_g = deg_new.reshape(NCORES, G, P).max(axis=(0, 2)).astype(np.int64)  # [G]

    # batch plan: consecutive groups at uniform pitch Smax = max S_g in the
    # batch, B*Smax <= SLOT_CAP, no batch crosses GS. Fetches stay exactly
    # S_g wide per group; the gbuf gap [S_g, Smax) is zeroed on device.
    batches = []           # (g0, B, Smax)
    g0 = 0
    while g0 < G:
        lim = GS if g0 < GS else G
        B = 1
        S = int(S_g[g0])
        while g0 + B < lim:
            S2 = max(S, int(S_g[g0 + B]))
            if (B + 1) * S2 > SLOT_CAP:
                break
            B += 1
            S = S2
        batches.append((g0, B, S))
        g0 += B
    S_list = [int(s) for s in S_g]
    total_cols = int(S_g.sum())

    # dummy zero row: core 0's last pad slot (deg 0 -> value always 0)
    dummy = int(_table_row(np.int64(0), np.int64(G - 1), np.int64(P - 1)))

    cum = np.concatenate([[0], np.cumsum(deg_new)]).astype(np.int64)
    S_max = int(S_g.max())
    jj = np.arange(S_max, dtype=np.int64)[None, :]
    pos = np.minimum(cum[:-1][:, None] + jj, len(s_sorted) - 1)
    valid = jj < deg_new[:, None]
    big = np.where(valid, s_sorted[pos], dummy).astype(np.int32)  # [NPAD, S_max]

    deg_f = deg_new.astype(np.float64)
    with np.errstate(divide="ignore"):
        dinv_all = np.where(deg_new > 0, 1.0 / np.sqrt(np.maximum(deg_f, 1e-12)), 0.0)
        dinv2_all = np.where(deg_new > 0, 1.0 / np.maximum(deg_f, 1e-12), 0.0)
        sqd_all = np.where(deg_new > 0, np.sqrt(deg_f), 0.0)

    bf16 = ml_dtypes.bfloat16
    idx_blobs, xts, dinvs, dinv2xs, sqdxs = [], [], [], [], []
    for c in range(NCORES):
        rows = slice(c * PC, (c + 1) * PC)
        bc = big[rows].reshape(G, P, S_max)
        blob = np.empty((P, total_cols), dtype=np.int32)
        off = 0
        for g in range(G):
            Sg = S_list[g]
            blob[:, off:off + Sg] = bc[g, :, :Sg]
            off += Sg
        idx_blobs.append(np.ascontiguousarray(blob))

        own_old = order[ranks[core_r == c]]          # old ids, local order asc
        xt = np.zeros((F_IN, PC), dtype=np.float32)
        xt[:, : len(own_old)] = x[own_old].T
        xts.append(np.ascontiguousarray(xt.astype(np.float16)))

        dinvs.append(np.ascontiguousarray(
            dinv_all[rows].reshape(G, P).T.astype(np.float32)))   # [128, G]
        d2 = dinv2_all[rows].reshape(G, P).T.astype(np.float16)   # [128, G]
        dinv2xs.append(np.ascontiguousarray(np.repeat(d2, C, axis=1)))
        sq = sqd_all[rows].reshape(G, P).T.astype(np.float16)
        sqdxs.append(np.ascontiguousarray(np.repeat(sq, C, axis=1)))

    w1t = np.ascontiguousarray(w1.T.astype(np.float16))    # [512, 256]
    w2t = np.ascontiguousarray(w2.T.astype(np.float16))    # [256, 64]

    return (new_id, batches, S_list, total_cols, idx_blobs, xts,
            dinvs, dinv2xs, sqdxs, w1t, w2t)


def _build_program(batches, S_list, total_cols, temps):
    import concourse.bass as bass
    import concourse.bacc as bacc
    import concourse.mybir as mybir
    import concourse.tile as tile
    from concourse.bass import broadcast_tensor_aps
    from concourse.masks import make_identity

    f32 = mybir.dt.float32
    f16 = mybir.dt.float16
    bf16 = mybir.dt.bfloat16
    i32 = mybir.dt.int32
    AF = mybir.ActivationFunctionType
    ALU = mybir.AluOpType

    maxB = max(B for (_, B, _) in batches)
    maxSB = max(B * S for (_, B, S) in batches)

    nc = bacc.Bacc(None, num_devices=NCORES)

    xt_d = nc.dram_tensor("xt", [F_IN, PC], f16, kind="ExternalInput")
    w1t_d = nc.dram_tensor("w1t", [F_IN, H], f16, kind="ExternalInput")
    b1_d = nc.dram_tensor("b1", [H], f32, kind="ExternalInput")
    w2t_d = nc.dram_tensor("w2t", [H, C], f16, kind="ExternalInput")
    b2_d = nc.dram_tensor("b2", [C], f32, kind="ExternalInput")
    dinv_d = nc.dram_tensor("dinv", [P, G], f32, kind="ExternalInput")
    dinv2x_d = nc.dram_tensor("dinv2x", [P, G * C], f16, kind="ExternalInput")
    sqdx_d = nc.dram_tensor("sqdx", [P, G * C], f16, kind="ExternalInput")
    idx_d = nc.dram_tensor("idx", [P, total_cols], i32, kind="ExternalInput")
    outl_d = nc.dram_tensor("outl", [P, G * C], f32, kind="ExternalOutput")

    own0_d = nc.dram_tensor("own0", [P, GS * C], f16)
    own1_d = nc.dram_tensor("own1", [P, G1 * C], f16)
    ha_d = nc.dram_tensor("ha", [NPAD, C], f16, addr_space="Shared")
    hb_d = nc.dram_tensor("hb", [NPAD, C], f16, addr_space="Shared")
    debug_dump = os.environ.get("KERNEL_DEBUG_DUMP", "0") == "1"
    if debug_dump:
        hdbg0_d = nc.dram_tensor("hdbg0", [NPAD, C], f16, kind="ExternalOutput")
        hdbg1_d = nc.dram_tensor("hdbg1", [NPAD, C], f16, kind="ExternalOutput")
    debug_gbuf = os.environ.get("KERNEL_DEBUG_GBUF", "0") == "1"
    if debug_gbuf:
        sb0 = batches[0][1] * batches[0][2] * C
        sbL = batches[-1][1] * batches[-1][2] * C
        gdbg0_d = nc.dram_tensor("gdbg0", [P, sb0], f16, kind="ExternalOutput")
        gdbgL_d = nc.dram_tensor("gdbgL", [P, sbL], f16, kind="ExternalOutput")

    groups = [list(range(NCORES))]

    with tile.TileContext(nc) as tc:
        with (
            tc.tile_pool(name="const", bufs=1) as cpool,
            tc.tile_pool(name="xin", bufs=3) as xpool,
            tc.tile_pool(name="mlp", bufs=3) as mpool,
            tc.tile_pool(name="small", bufs=4) as spool,
            tc.tile_pool(name="ps", bufs=2, space="PSUM") as ppool,
            tc.tile_pool(name="ps2", bufs=2, space="PSUM") as ppool2,
        ):
            # ---- constants / persistent state ----
            w1t_sb = cpool.tile([P, 4 * H], f16)     # [128, (kc, 256)]
            nc.sync.dma_start(
                w1t_sb[:].rearrange("p (kc h) -> p kc h", kc=4),
                w1t_d[:].rearrange("(kc p) h -> p kc h", p=P))
            w2t_sb = cpool.tile([P, 2 * C], f16)     # [128, (jc, 64)]
            nc.sync.dma_start(
                w2t_sb[:].rearrange("p (jc c) -> p jc c", jc=2),
                w2t_d[:].rearrange("(jc p) c -> p jc c", p=P))
            b1_sb = cpool.tile([P, 2], f32)
            nc.sync.dma_start(b1_sb[:], b1_d[:].rearrange("(jc p) -> p jc", p=P))
            b2_sb = cpool.tile([P, 1], f32)
            nc.sync.dma_start(b2_sb[:C, :], b2_d[:].rearrange("(c one) -> c one", one=1))
            dinv_sb = cpool.tile([P, G], f32)
            nc.sync.dma_start(dinv_sb[:], dinv_d[:])
            dinv2x_sb = cpool.tile([P, G * C], f16)
            nc.sync.dma_start(dinv2x_sb[:], dinv2x_d[:])
            sqdx_sb = cpool.tile([P, G * C], f16)
            nc.sync.dma_start(sqdx_sb[:], sqdx_d[:])
            idx_sb = cpool.tile([P, total_cols], i32)
            nc.sync.dma_start(idx_sb[:], idx_d[:])
            ident = cpool.tile([P, P], f32)
            make_identity(nc, ident[:])
            hidden = cpool.tile([P, G * C], f16)
            gall = cpool.tile([P, G * C], f16)

            def flush_chunk(chunk, dst_table):
                if chunk == 0:
                    nc.sync.dma_start(own0_d[:], gall[:, : GS * C])
                    nc.gpsimd.collective_compute(
                        "AllGather", ALU.bypass, replica_groups=groups,
                        ins=[own0_d[:]], outs=[dst_table[0:ROWS0, :]])
                else:
                    nc.sync.dma_start(own1_d[:], gall[:, GS * C:])
                    nc.gpsimd.collective_compute(
                        "AllGather", ALU.bypass, replica_groups=groups,
                        ins=[own1_d[:]], outs=[dst_table[ROWS0:, :]])

            # ---- phase A: MLP + g0 ----
            for g in range(G):
                xt_sb = xpool.tile([P, 4, P], f16, tag="xt")
                nc.sync.dma_start(
                    xt_sb[:],
                    xt_d[:, g * P:(g + 1) * P].rearrange(
                        "(kc p) n -> p kc n", p=P))
                h1_sb = mpool.tile([P, 2 * P], f16, tag="h1")
                for jc in range(2):
                    ps1 = ppool.tile([P, P], f32, tag="ps1")
                    for kc in range(4):
                        nc.tensor.matmul(
                            ps1[:],
                            lhsT=w1t_sb[:, kc * H + jc * P: kc * H + (jc + 1) * P],
                            rhs=xt_sb[:, kc, :],
                            start=(kc == 0), stop=(kc == 3))
                    nc.scalar.activation(
                        h1_sb[:, jc * P:(jc + 1) * P], ps1[:],
                        AF.Relu, bias=b1_sb[:, jc:jc + 1])
                ps2 = ppool.tile([P, P], f32, tag="ps2")
                for jc in range(2):
                    nc.tensor.matmul(
                        ps2[:C, :],
                        lhsT=w2t_sb[:, jc * C:(jc + 1) * C],
                        rhs=h1_sb[:, jc * P:(jc + 1) * P],
                        start=(jc == 0), stop=(jc == 1))
                h2_sb = mpool.tile([P, P], f32, tag="h2")
                nc.scalar.activation(h2_sb[:C, :], ps2[:C, :],
                                     AF.Identity, bias=b2_sb[:C, :])
                pst = ppool2.tile([P, C], f32, tag="pst")
                nc.tensor.transpose(pst[:], h2_sb[:C, :], ident[:C, :C])
                sl = slice(g * C, (g + 1) * C)
                nc.vector.tensor_scalar_mul(gall[:, sl], pst[:], dinv_sb[:, g:g + 1])
                nc.scalar.mul(hidden[:, sl], gall[:, sl], float(temps[0]))
                if g == GS - 1:
                    flush_chunk(0, ha_d)
            flush_chunk(1, ha_d)
            if debug_dump:
                nc.sync.dma_start(hdbg0_d[:], ha_d[:])

            # ---- phase B: K hops ----
            with tc.tile_pool(name="gat", bufs=4) as gpool:
                hcur, hnxt = ha_d, hb_d
                for k in range(K):
                    tk = float(temps[k + 1])
                    off = 0
                    for (g0, B, S) in batches:
                        gbuf = gpool.tile([P, maxSB * C], f16, tag="gbuf")
                        for b in range(B):
                            Sb = S_list[g0 + b]
                            nc.gpsimd.indirect_dma_start(
                                out=gbuf[:, b * S * C: b * S * C + Sb * C],
                                out_offset=None,
                                in_=hcur[:],
                                in_offset=bass.IndirectOffsetOnAxis(
                                    ap=idx_sb[:, off: off + Sb], axis=0))
                            if Sb < S:
                                nc.vector.memset(
                                    gbuf[:, b * S * C + Sb * C:
                                         (b + 1) * S * C], 0.0)
                            off += Sb
                        # log-tree fold over slots (group-major layout, strided)
                        gv = gbuf[:, : B * S * C].rearrange(
                            "p (b x) -> p b x", b=B)
                        s = S
                        while s > 1:
                            h_ = s // 2
                            nc.vector.tensor_add(
                                gv[:, :, : h_ * C],
                                gv[:, :, : h_ * C],
                                gv[:, :, (s - h_) * C: s * C])
                            s -= h_
                        sl = slice(g0 * C, (g0 + B) * C)
                        nc.vector.tensor_tensor(
                            out=gall[:, sl].rearrange("p (b c) -> p b c", c=C),
                            in0=gv[:, :, :C],
                            in1=dinv2x_sb[:, sl].rearrange(
                                "p (b c) -> p b c", c=C),
                            op=ALU.mult)
                        tmp = spool.tile([P, maxB * C], f16, tag="tmp")
                        nc.scalar.mul(tmp[:, : B * C], gall[:, sl], tk)
                        nc.vector.tensor_add(
                            hidden[:, sl], hidden[:, sl], tmp[:, : B * C])
                        if k < K - 1:
                            if g0 + B == GS:
                                flush_chunk(0, hnxt)
                            elif g0 + B == G:
                                flush_chunk(1, hnxt)
                    if debug_dump and k == 0:
                        nc.sync.dma_start(hdbg1_d[:], hnxt[:])
                    hcur, hnxt = hnxt, hcur

            # ---- phase C: hidden * sqrt(deg), log_softmax, store ----
            with tc.tile_pool(name="smx", bufs=1) as opool:
                hidf = opool.tile([P, G * C], f32)
                nc.vector.tensor_tensor(
                    out=hidf[:], in0=hidden[:], in1=sqdx_sb[:], op=ALU.mult)
                hid3 = hidf[:].rearrange("p (g c) -> p g c", c=C)
                nm = opool.tile([P, G], f32)
                nc.vector.reduce_max(nm[:], hid3, axis=mybir.AxisListType.X,
                                     negate=True)
                nm3 = nm[:].rearrange("p (g one) -> p g one", one=1)
                h_b, nm_b = broadcast_tensor_aps(hid3, nm3)
                nc.vector.tensor_tensor(out=hid3, in0=h_b, in1=nm_b, op=ALU.add)
                exf = opool.tile([P, G * C], f16)
                nc.scalar.activation(exf[:], hidf[:], AF.Exp)
                ssum = opool.tile([P, G], f32)
                nc.vector.reduce_sum(
                    ssum[:], exf[:].rearrange("p (g c) -> p g c", c=C),
                    axis=mybir.AxisListType.X)
                lse = opool.tile([P, G], f32)
                nc.scalar.activation(lse[:], ssum[:], AF.Ln)
                lse3 = lse[:].rearrange("p (g one) -> p g one", one=1)
                osb = opool.tile([P, G * C], f32)
                h_b2, lse_b = broadcast_tensor_aps(hid3, lse3)
                nc.vector.tensor_tensor(
                    out=osb[:].rearrange("p (g c) -> p g c", c=C),
                    in0=h_b2, in1=lse_b, op=ALU.subtract)
                nc.sync.dma_start(outl_d[:], osb[:])

    nc.finalize()
    return nc


def kernel(x, w1, b1, w2, b2, temp, edge_index):
    from concourse.bass_utils import run_bass_kernel_spmd

    x = np.asarray(x, dtype=np.float32)
    w1 = np.asarray(w1, dtype=np.float32)
    b1 = np.asarray(b1, dtype=np.float32)
    w2 = np.asarray(w2, dtype=np.float32)
    b2 = np.asarray(b2, dtype=np.float32)
    temp = np.asarray(temp, dtype=np.float32)

    (new_id, batches, S_list, total_cols, idx_blobs, xts,
     dinvs, dinv2xs, sqdxs, w1t, w2t) = _host_prep(x, w1, w2, edge_index)

    nc = _build_program(batches, S_list, total_cols, [float(t) for t in temp])

    in_maps = []
    for c in range(NCORES):
        in_maps.append({
            "xt": xts[c],
            "w1t": w1t, "b1": b1, "w2t": w2t, "b2": b2,
            "dinv": dinvs[c], "dinv2x": dinv2xs[c], "sqdx": sqdxs[c],
            "idx": idx_blobs[c],
        })

    trace = os.environ.get("KERNEL_TRACE", "0") == "1"
    res = run_bass_kernel_spmd(nc, in_maps, list(range(NCORES)), trace=trace)
    if trace:
        _profile_info["exec_time_ns"] = res.exec_time_ns
        _profile_info["mean_exec_time_ns"] = res.mean_exec_time_ns
        _profile_info["profile_json"] = res.profile_json

    # outl is [P, G*C] partition-major; node (core, g, p) -> [p, g*C:(g+1)*C]
    parts = []
    for c in range(NCORES):
        o = res.results[c]["outl"].reshape(P, G, C)
        parts.append(np.ascontiguousarray(o.transpose(1, 0, 2).reshape(PC, C)))
    full = np.concatenate(parts, axis=0)
    return np.ascontiguousarray(full[new_id])
